# revision 53
# baseline (speedup 1.0000x reference)
"""Trainium2 Bass kernel: weighted-KDE avoid-distance (retrieval_knn).

dist[n] = mean_m exp(-0.5 * sum_d (means[m,d]-samples[n,d])^2 / stds[m,d])
out     = -dist + max(dist) + min(dist)

Data-parallel over N=8192 samples across 8 cores (1024 each; full means/stds
per core).

Single fp32r matmul pass (PE rounds operands to ~fp22; 1 cycle/row at
>=256 moving cols — same rate as bf16, HALF the passes of the old bf16
hi/lo scheme, and no hi/lo feature packing):
  logp[n,m] = sB.mB + s2.wf + ones.mq      (K = 96 rows of 32)
    sB = -2s (Act), s2 = s^2 (Act Square), ones = 1 (memset at t0)
    r = 1/std (DVE approx recip), wf = -0.5r, mB = m*wf, mq = m*mB (DVE)
  walrus requires fp32r matmul inputs to be *produced* as fp32r, so the
  PSUM->SBUF staging copies write f32r tiles (the copy rounds); the
  transposes stay plain f32.  Measured HW rel err 2.1e-3 vs the 2e-2
  gate (numpy sim of fp22 truncation: 5.2e-3; single-pass bf16: 0.46).

Single-shot-latency structure (timed against TimelineSim, whose
per-tile access chains and DMA latency model match the scheduler):
  - stds (SP) / means (Act) DMA'd unsplit: HWDGE descriptor processing
    is ~625ns per transfer and serialized, so fewer transfers reach the
    last-needed tensor sooner; each DMA pays desc(625) + DGE(650) +
    transfer + 900ns semaphore propagation
  - samples ride the Pool SWDGE (separate descriptor path), chunk 0
    split out so the chunk-0 lhs transpose can go early
  - dummy PE transposes from t~1us ride out the clock-gate ramp
  - [128,1] exp at t0 preloads the activation table set off the path
  - feature chain on DVE (recip -> wf -> mB -> mq) split in HALVES with
    separate r/packed tiles (DMAs stay unsplit), so half-A's transposes
    start ~0.9us earlier and half-B's features overlap them; sample
    features on Act; all 24 transposes (f32, PE) land in PSUM slots.  Scheduling
    rule: every access to a tile serializes with all other accesses to
    that tile (read-read too), so the staging SPLITS across both PSUM
    tile chains: samples -> T1 slots 8-15 (s1T drains on DVE+Act), rhs
    tiles 0-7 -> T0 slots 0-7 (over the warmup junk; rhs01 drain on
    DVE), rhs tiles 8-15 -> T1 slots 0-7 (rhs23 drain on DVE) — the two
    rhs drains ride separate chains and overlap
  - main loop: 4 fp32r matmuls (512 cols) per chunk; exp in full
    [128,2048] chunks; chunk 0 split in halves ON SEPARATE PSUM buffers
    (half A in T0 cols 0:1024, half B in T1 cols 1024:2048) so half B's
    matmuls never sit behind half A's exp read in the same tile chain
  - reduce split: chunks 5-7 use ScalarE accum_out (187ns aux each),
    chunks 0-4 fp32 eo + DVE tensor_reduce (DVE is idle in main loop;
    GpSimd cannot access PSUM so Pool stays off the drain/reduce paths)
  - output DMA split on SP: bulk piece [*,0:7] goes while chunk 7
    finishes, final element alone pays the post-transfer latency

Final -dist+max+min flip on host after gathering shards.
"""

import sys

import numpy as np

for _p in ("/opt/trn_rl_repo", "/root/.axon_site/_ro/trn_rl_repo"):
    if _p not in sys.path:
        sys.path.insert(0, _p)

N, M, D = 8192, 2048, 32
N_CORES = 8
NSH = N // N_CORES        # 1024 samples per core
MT = M // 128             # 16 mean tiles
CT = NSH // 128           # 8 sample chunks per core
K = 96                    # contraction rows: [sB(32), s2(32), ones(32)]
LN_M = float(np.log(M))   # ln(2048); exp bias folds the 1/M mean

N_WARM = 14               # dummy transposes riding out the PE clock ramp

_CACHE = {}


def _build_nc(reps: int = 1):
    import concourse.bacc as bacc
    import concourse.tile as tile
    from concourse import mybir
    from concourse.masks import make_identity

    f32 = mybir.dt.float32
    f32r = mybir.dt.float32r
    bf16 = mybir.dt.bfloat16
    AF = mybir.ActivationFunctionType
    OP = mybir.AluOpType
    AX = mybir.AxisListType

    nc = bacc.Bacc("TRN2", target_bir_lowering=False, debug=False)

    samples_d = nc.dram_tensor("samples", [NSH, D], f32, kind="ExternalInput")
    means_d = nc.dram_tensor("means", [M, D], f32, kind="ExternalInput")
    stds_d = nc.dram_tensor("stds", [M, D], f32, kind="ExternalInput")
    dist_d = nc.dram_tensor("dist", [NSH], f32, kind="ExternalOutput")

    with tile.TileContext(nc) as tc:
        with (
            tc.tile_pool(name="persist", bufs=1) as pp,
            tc.tile_pool(name="psum", bufs=2, space="PSUM") as psp,
            tc.tile_pool(name="expo", bufs=3) as xp,
        ):
          for _rep in range(reps):
            # ---- t0: PE warmup operand first so the clock ramp starts asap
            garb = pp.tile([128, 128], bf16)
            nc.vector.memset(garb[:], 0.0)
            # input DMA: stds (SP), means (Act) unsplit — HWDGE descriptor
            # processing is ~625ns per transfer regardless of size and fully
            # serialized, so fewer transfers reach the last-needed tensor
            # (means) sooner. Samples ride the Pool SWDGE (separate desc
            # path), chunk 0 split out so its lhs chain starts early.
            stds_nat = pp.tile([128, MT, D], f32)
            means_nat = pp.tile([128, MT, D], f32)
            samp_c0 = pp.tile([128, 1, D], f32)
            samp_nat = pp.tile([128, CT - 1, D], f32)
            stds_ap = stds_d.ap().rearrange("(p t) d -> p t d", p=128)
            means_ap = means_d.ap().rearrange("(p t) d -> p t d", p=128)
            samp_ap = samples_d.ap().rearrange("(p c) d -> p c d", p=128)
            nc.sync.dma_start(stds_nat[:], stds_ap[:])
            nc.scalar.dma_start(means_nat[:], means_ap[:])
            nc.gpsimd.dma_start(samp_c0[:], samp_ap[:, 0:1])
            nc.gpsimd.dma_start(samp_nat[:], samp_ap[:, 1:CT])

            scr0 = pp.tile([128, 1], f32)
            nc.vector.memset(scr0[:], 0.0)
            scr1 = pp.tile([128, 1], f32)
            # preload the exp table set while DMA/features run
            nc.scalar.activation(scr1[:], scr0[:], AF.Exp)
            ebias = pp.tile([128, 1], f32)
            nc.vector.memset(ebias[:], -LN_M)
            spack0 = pp.tile([128, 1, K], f32)
            spackR = pp.tile([128, CT - 1, K], f32)
            nc.vector.memset(spack0[:, :, 64:96], 1.0)    # ones rows
            nc.vector.memset(spackR[:, :, 64:96], 1.0)
            identity = pp.tile([128, 128], f32)
            make_identity(nc, identity[:])                # Pool

            # ---- PSUM slot plan (f32 [*,128] slots in the two mm buffers)
            # T1 slots 0-7: rhs tiles 0-7; T1 slots 8-15: sample tiles 0-7
            # T0 slots 8-15: rhs tiles 8-15
            T0 = psp.tile([128, M], f32, tag="mm")
            T1 = psp.tile([128, M], f32, tag="mm")

            def slot(tb, k):
                return tb[:, k * 128:(k + 1) * 128]

            T0b = T0.bitcast(bf16)
            for i in range(N_WARM):
                nc.tensor.transpose(T0b[:, (i % 8) * 128:(i % 8) * 128 + 128], garb[:], garb[:])

            # ---- features ----
            # samples (Act): sB = -2s, s2 = s^2; chunk 0 split out so its
            # transpose can go as soon as the tiny chunk-0 DMA lands
            nc.scalar.mul(spack0[:, :, 0:D], samp_c0[:], -2.0)
            nc.scalar.activation(spack0[:, :, D:2 * D], samp_c0[:], AF.Square)

            # means/stds feature chain on DVE, split in HALVES with separate
            # tiles (DMAs stay unsplit): half-A's chain completes at
            # means-sem + 2 half-ops instead of + 2 full-ops, so the rhs0-7
            # transposes/drain start ~0.9us earlier; half-B's feature ops
            # overlap half-A's transposes.
            HT = MT // 2
            r_h = [pp.tile([128, HT, D], f32, name=f"r{h}") for h in (0, 1)]
            pk_h = [pp.tile([128, HT, K], f32, name=f"pk{h}") for h in (0, 1)]
            nc.scalar.mul(spackR[:, :, 0:D], samp_nat[:], -2.0)
            nc.scalar.activation(spackR[:, :, D:2 * D], samp_nat[:], AF.Square)
            for h, hh in ((0, slice(0, HT)), (1, slice(HT, MT))):
                nc.vector.reciprocal_approx_fast(r_h[h][:], stds_nat[:, hh])
                nc.vector.tensor_scalar_mul(pk_h[h][:, :, D:2 * D], r_h[h][:], -0.5)
                nc.vector.scalar_tensor_tensor(                       # mB
                    pk_h[h][:, :, 0:D], means_nat[:, hh], -0.5, r_h[h][:],
                    op0=OP.mult, op1=OP.mult)
                nc.vector.scalar_tensor_tensor(                       # mq
                    pk_h[h][:, :, 2 * D:3 * D], means_nat[:, hh], 1.0,
                    pk_h[h][:, :, 0:D], op0=OP.mult, op1=OP.mult)

            # ---- transposes (PE, f32) + staging copies ----
            # Slot plan (every access to a tile serializes with all other
            # accesses to it, so per-tile chains are kept short and copies
            # come after all transposes of their source tile):
            #   T0 slots 8-15: sample tiles 0-7 (drained by DVE+Act s1T copies)
            #   T1 slots 0-15: rhs tiles 0-15 (drained by DVE+Act copies)
            # The copies write f32r tiles — walrus requires fp32r matmul
            # inputs to be produced as fp32r (the copy does the rounding).
            s1T = pp.tile([128, NSH], f32r)
            rhs01 = pp.tile([128, 1024], f32r)
            rhs23 = pp.tile([128, 1024], f32r)

            def tp(dst_slot, src):
                nc.tensor.transpose(dst_slot[0:K], src, identity[:])

            dist_sb = pp.tile([128, CT], f32)

            def mm(ps, c, j):
                rtile = rhs01 if j < 2 else rhs23
                nc.tensor.matmul(
                    ps[:, j * 512:(j + 1) * 512],
                    lhsT=s1T[0:K, c * 128:(c + 1) * 128],
                    rhs=rtile[0:K, (j % 2) * 512:(j % 2) * 512 + 512],
                    start=True, stop=True, skip_group_check=True)

            tp(slot(T1, 8), spack0[:, 0, :])             # sample tile 0 first
            nc.vector.tensor_copy(s1T[0:K, 0:128], T1[0:K, 1024:1152])
            for t in range(8):                           # rhs tiles 0-7 -> T0 s0-7
                tp(slot(T0, t), pk_h[0][:, t, :])        # (over warmup junk)
            nc.vector.tensor_copy(rhs01[0:K, :], T0[0:K, 0:1024])
            for c in range(1, CT):                       # sample tiles 1-7 -> T1 s9-15
                tp(slot(T1, 8 + c), spackR[:, c - 1, :])
            for t in range(8):                           # rhs tiles 8-15 -> T1 s0-7
                tp(slot(T1, t), pk_h[1][:, t, :])
            # rhs drains ride two separate tile chains and overlap
            nc.vector.tensor_copy(rhs23[0:K, :], T1[0:K, 0:1024])
            nc.scalar.copy(s1T[0:K, 128:1024], T1[0:K, 1152:2048])

            # ---- main loop; chunk 0 split in halves so its first exp
            # starts as soon as rhs01 lands (the per-tile access chain
            # serializes mm/exp interleave safely in program order) ----
            dh = pp.tile([128, 2], f32)
            # chunk 0 halves on DIFFERENT PSUM buffers: half A in T0 cols
            # 0:1024, half B in T1 cols 1024:2048 (sample slots, long since
            # drained) — so expB's matmuls never sit behind expA's read in
            # the same tile chain.
            psA = psp.tile([128, M], f32, tag="mm")
            psB = psp.tile([128, M], f32, tag="mm")
            eo0 = xp.tile([128, M], f32, tag="eof", name="eof")
            mm(psA, 0, 0)
            mm(psA, 0, 1)
            nc.scalar.activation(eo0[:, 0:1024], psA[:, 0:1024], AF.Exp,
                                 bias=ebias[:], scale=1.0)
            nc.tensor.matmul(psB[:, 1024:1536], lhsT=s1T[0:K, 0:128],
                             rhs=rhs23[0:K, 0:512],
                             start=True, stop=True, skip_group_check=True)
            nc.tensor.matmul(psB[:, 1536:2048], lhsT=s1T[0:K, 0:128],
                             rhs=rhs23[0:K, 512:1024],
                             start=True, stop=True, skip_group_check=True)
            nc.scalar.activation(eo0[:, 1024:2048], psB[:, 1024:2048],
                                 AF.Exp, bias=ebias[:], scale=1.0)
            nc.vector.tensor_reduce(dh[:, 0:1], eo0[:, 0:1024],
                                    axis=AX.X, op=OP.add)
            nc.vector.tensor_reduce(dh[:, 1:2], eo0[:, 1024:2048],
                                    axis=AX.X, op=OP.add)
            nc.vector.tensor_reduce(dist_sb[:, 0:1], dh[:], axis=AX.X,
                                    op=OP.add)
            for c in range(1, CT):
                ps = psp.tile([128, M], f32, tag="mm")
                if c <= 4:
                    eo = xp.tile([128, M], f32, tag="eof", name="eof")
                else:
                    eo = xp.tile([128, M], bf16, tag="eo", name="eo")
                for j in range(4):
                    mm(ps, c, j)
                if c <= 4:
                    nc.scalar.activation(eo[:], ps[:], AF.Exp, bias=ebias[:],
                                         scale=1.0)
                    nc.vector.tensor_reduce(dist_sb[:, c:c + 1], eo[:],
                                            axis=AX.XY, op=OP.add)
                else:
                    nc.scalar.activation(eo[:], ps[:], AF.Exp, bias=ebias[:],
                                         scale=1.0, accum_out=dist_sb[:, c:c + 1])

            # output DMA split on SP: the bulk piece overlaps chunk 7's exp,
            # only the final element pays the post-transfer latency
            dist_ap = dist_d.ap().rearrange("(p c) -> p c", p=128)
            nc.sync.dma_start(dist_ap[:, 0:CT - 1], dist_sb[:, 0:CT - 1])
            nc.sync.dma_start(dist_ap[:, CT - 1:CT], dist_sb[:, CT - 1:CT])

    nc.compile()
    return nc


def _get_nc():
    if "nc" not in _CACHE:
        _CACHE["nc"] = _build_nc()
    return _CACHE["nc"]


def kernel(samples: np.ndarray, means: np.ndarray, stds: np.ndarray) -> np.ndarray:
    from concourse.bass_utils import run_bass_kernel_spmd

    samples = np.ascontiguousarray(samples, dtype=np.float32)
    means = np.ascontiguousarray(means, dtype=np.float32)
    stds = np.ascontiguousarray(stds, dtype=np.float32)

    nc = _get_nc()
    in_maps = [
        {"samples": samples[i * NSH:(i + 1) * NSH], "means": means, "stds": stds}
        for i in range(N_CORES)
    ]
    res = run_bass_kernel_spmd(nc, in_maps, list(range(N_CORES)))
    dist = np.concatenate([res.results[i]["dist"] for i in range(N_CORES)])
    return (-dist + dist.max() + dist.min()).astype(np.float32)


# revision 58
# speedup vs baseline: 1.0004x; 1.0004x over previous
"""Trainium2 Bass kernel: weighted-KDE avoid-distance (retrieval_knn).

dist[n] = mean_m exp(-0.5 * sum_d (means[m,d]-samples[n,d])^2 / stds[m,d])
out     = -dist + max(dist) + min(dist)

Data-parallel over N=8192 samples across 8 cores (1024 each; full means/stds
per core).

Single fp32r matmul pass (PE rounds operands to ~fp22; 1 cycle/row at
>=256 moving cols — same rate as bf16, HALF the passes of the old bf16
hi/lo scheme, and no hi/lo feature packing):
  logp[n,m] = sB.mB + s2.wf + ones.mq      (K = 96 rows of 32)
    sB = -2s (Act), s2 = s^2 (Act Square), ones = 1 (memset at t0)
    r = 1/std (DVE approx recip), wf = -0.5r, mB = m*wf, mq = m*mB (DVE)
  walrus requires fp32r matmul inputs to be *produced* as fp32r, so the
  PSUM->SBUF staging copies write f32r tiles (the copy rounds); the
  transposes stay plain f32.  Measured HW rel err 2.1e-3 vs the 2e-2
  gate (numpy sim of fp22 truncation: 5.2e-3; single-pass bf16: 0.46).

Single-shot-latency structure (timed against TimelineSim, whose
per-tile access chains and DMA latency model match the scheduler):
  - stds (SP) / means (Act) DMA'd unsplit: HWDGE descriptor processing
    is ~625ns per transfer and serialized, so fewer transfers reach the
    last-needed tensor sooner; each DMA pays desc(625) + DGE(650) +
    transfer + 900ns semaphore propagation
  - samples ride the Pool SWDGE (separate descriptor path), chunk 0
    split out so the chunk-0 lhs transpose can go early
  - dummy PE transposes from t~1us ride out the clock-gate ramp
  - [128,1] exp at t0 preloads the activation table set off the path
  - feature chain on DVE (recip -> wf -> mB -> mq) split in HALVES with
    separate r/packed tiles (DMAs stay unsplit), so half-A's transposes
    start ~0.9us earlier and half-B's features overlap them; sample
    features on Act; all 24 transposes (f32, PE) land in PSUM slots.  Scheduling
    rule: every access to a tile serializes with all other accesses to
    that tile (read-read too), so the staging SPLITS across both PSUM
    tile chains: samples -> T1 slots 8-15 (s1T drains on DVE+Act), rhs
    tiles 0-7 -> T0 slots 0-7 (over the warmup junk; rhs01 drain on
    DVE), rhs tiles 8-15 -> T1 slots 0-7 (rhs23 drain on DVE) — the two
    rhs drains ride separate chains and overlap
  - main loop: 4 fp32r matmuls (512 cols) per chunk; exp in full
    [128,2048] chunks; chunk 0 split in halves ON SEPARATE PSUM buffers
    (half A in T0 cols 0:1024, half B in T1 cols 1024:2048) so half B's
    matmuls never sit behind half A's exp read in the same tile chain
  - reduce split: chunks 5-7 use ScalarE accum_out (187ns aux each),
    chunks 0-4 fp32 eo + DVE tensor_reduce (DVE is idle in main loop;
    GpSimd cannot access PSUM so Pool stays off the drain/reduce paths)
  - output DMA split on SP: bulk piece [*,0:7] goes while chunk 7
    finishes, final element alone pays the post-transfer latency

Final -dist+max+min flip on host after gathering shards.
"""

import sys

import numpy as np

for _p in ("/opt/trn_rl_repo", "/root/.axon_site/_ro/trn_rl_repo"):
    if _p not in sys.path:
        sys.path.insert(0, _p)

N, M, D = 8192, 2048, 32
N_CORES = 8
NSH = N // N_CORES        # 1024 samples per core
MT = M // 128             # 16 mean tiles
CT = NSH // 128           # 8 sample chunks per core
K = 96                    # contraction rows: [sB(32), s2(32), ones(32)]
LN_M = float(np.log(M))   # ln(2048); exp bias folds the 1/M mean

N_WARM = 14               # dummy transposes riding out the PE clock ramp

_CACHE = {}


def _build_nc(reps: int = 1):
    import concourse.bacc as bacc
    import concourse.tile as tile
    from concourse import mybir
    from concourse.masks import make_identity

    f32 = mybir.dt.float32
    f32r = mybir.dt.float32r
    bf16 = mybir.dt.bfloat16
    AF = mybir.ActivationFunctionType
    OP = mybir.AluOpType
    AX = mybir.AxisListType

    nc = bacc.Bacc("TRN2", target_bir_lowering=False, debug=False)

    samples_d = nc.dram_tensor("samples", [NSH, D], f32, kind="ExternalInput")
    means_d = nc.dram_tensor("means", [M, D], f32, kind="ExternalInput")
    stds_d = nc.dram_tensor("stds", [M, D], f32, kind="ExternalInput")
    dist_d = nc.dram_tensor("dist", [NSH], f32, kind="ExternalOutput")

    with tile.TileContext(nc) as tc:
        with (
            tc.tile_pool(name="persist", bufs=1) as pp,
            tc.tile_pool(name="psum", bufs=2, space="PSUM") as psp,
            tc.tile_pool(name="expo", bufs=3) as xp,
        ):
          for _rep in range(reps):
            # ---- t0: PE warmup operand first so the clock ramp starts asap
            garb = pp.tile([128, 128], bf16)
            nc.vector.memset(garb[:], 0.0)
            # input DMA: stds (SP), means (Act) unsplit — HWDGE descriptor
            # processing is ~625ns per transfer regardless of size and fully
            # serialized, so fewer transfers reach the last-needed tensor
            # (means) sooner. Samples ride the Pool SWDGE (separate desc
            # path), chunk 0 split out so its lhs chain starts early.
            stds_nat = pp.tile([128, MT, D], f32)
            means_nat = pp.tile([128, MT, D], f32)
            samp_c0 = pp.tile([128, 1, D], f32)
            samp_nat = pp.tile([128, CT - 1, D], f32)
            stds_ap = stds_d.ap().rearrange("(p t) d -> p t d", p=128)
            means_ap = means_d.ap().rearrange("(p t) d -> p t d", p=128)
            samp_ap = samples_d.ap().rearrange("(p c) d -> p c d", p=128)
            nc.sync.dma_start(stds_nat[:], stds_ap[:])
            nc.scalar.dma_start(means_nat[:], means_ap[:])
            nc.gpsimd.dma_start(samp_c0[:], samp_ap[:, 0:1])
            nc.gpsimd.dma_start(samp_nat[:], samp_ap[:, 1:CT])

            scr0 = pp.tile([128, 1], f32)
            nc.vector.memset(scr0[:], 0.0)
            scr1 = pp.tile([128, 1], f32)
            # preload the exp table set while DMA/features run
            nc.scalar.activation(scr1[:], scr0[:], AF.Exp)
            ebias = pp.tile([128, 1], f32)
            nc.vector.memset(ebias[:], -LN_M)
            spack0 = pp.tile([128, 1, K], f32)
            spackR = pp.tile([128, CT - 1, K], f32)
            nc.vector.memset(spack0[:, :, 64:96], 1.0)    # ones rows
            nc.vector.memset(spackR[:, :, 64:96], 1.0)
            identity = pp.tile([128, 128], f32)
            make_identity(nc, identity[:])                # Pool

            # ---- PSUM slot plan (f32 [*,128] slots in the two mm buffers)
            # T1 slots 0-7: rhs tiles 0-7; T1 slots 8-15: sample tiles 0-7
            # T0 slots 8-15: rhs tiles 8-15
            T0 = psp.tile([128, M], f32, tag="mm")
            T1 = psp.tile([128, M], f32, tag="mm")

            def slot(tb, k):
                return tb[:, k * 128:(k + 1) * 128]

            T0b = T0.bitcast(bf16)
            for i in range(N_WARM):
                nc.tensor.transpose(T0b[:, (i % 8) * 128:(i % 8) * 128 + 128], garb[:], garb[:])

            # ---- features ----
            # samples (Act): sB = -2s, s2 = s^2; chunk 0 split out so its
            # transpose can go as soon as the tiny chunk-0 DMA lands
            nc.scalar.mul(spack0[:, :, 0:D], samp_c0[:], -2.0)
            nc.scalar.activation(spack0[:, :, D:2 * D], samp_c0[:], AF.Square)

            # means/stds feature chain on DVE, split in HALVES with separate
            # tiles (DMAs stay unsplit): half-A's chain completes at
            # means-sem + 2 half-ops instead of + 2 full-ops, so the rhs0-7
            # transposes/drain start ~0.9us earlier; half-B's feature ops
            # overlap half-A's transposes.
            HT = MT // 2
            r_h = [pp.tile([128, HT, D], f32, name=f"r{h}") for h in (0, 1)]
            pk_h = [pp.tile([128, HT, K], f32, name=f"pk{h}") for h in (0, 1)]
            nc.scalar.mul(spackR[:, :, 0:D], samp_nat[:], -2.0)
            nc.scalar.activation(spackR[:, :, D:2 * D], samp_nat[:], AF.Square)
            for h, hh in ((0, slice(0, HT)), (1, slice(HT, MT))):
                nc.vector.reciprocal_approx_fast(r_h[h][:], stds_nat[:, hh])
                nc.vector.tensor_scalar_mul(pk_h[h][:, :, D:2 * D], r_h[h][:], -0.5)
                nc.vector.scalar_tensor_tensor(                       # mB
                    pk_h[h][:, :, 0:D], means_nat[:, hh], -0.5, r_h[h][:],
                    op0=OP.mult, op1=OP.mult)
                nc.vector.scalar_tensor_tensor(                       # mq
                    pk_h[h][:, :, 2 * D:3 * D], means_nat[:, hh], 1.0,
                    pk_h[h][:, :, 0:D], op0=OP.mult, op1=OP.mult)

            # ---- transposes (PE, f32) + staging copies ----
            # Slot plan (every access to a tile serializes with all other
            # accesses to it, so per-tile chains are kept short and copies
            # come after all transposes of their source tile):
            #   T0 slots 8-15: sample tiles 0-7 (drained by DVE+Act s1T copies)
            #   T1 slots 0-15: rhs tiles 0-15 (drained by DVE+Act copies)
            # The copies write f32r tiles — walrus requires fp32r matmul
            # inputs to be produced as fp32r (the copy does the rounding).
            s1T = pp.tile([128, NSH], f32r)
            rhs01 = pp.tile([128, 1024], f32r)
            rhs23 = pp.tile([128, 1024], f32r)

            def tp(dst_slot, src):
                nc.tensor.transpose(dst_slot[0:K], src, identity[:])

            dist_sb = pp.tile([128, CT], f32)

            def mm(ps, c, j):
                rtile = rhs01 if j < 2 else rhs23
                nc.tensor.matmul(
                    ps[:, j * 512:(j + 1) * 512],
                    lhsT=s1T[0:K, c * 128:(c + 1) * 128],
                    rhs=rtile[0:K, (j % 2) * 512:(j % 2) * 512 + 512],
                    start=True, stop=True, skip_group_check=True)

            tp(slot(T1, 8), spack0[:, 0, :])             # sample tile 0 first
            nc.vector.tensor_copy(s1T[0:K, 0:128], T1[0:K, 1024:1152])
            for t in range(8):                           # rhs tiles 0-7 -> T0 s0-7
                tp(slot(T0, t), pk_h[0][:, t, :])        # (over warmup junk)
            nc.vector.tensor_copy(rhs01[0:K, :], T0[0:K, 0:1024])
            for c in range(1, CT):                       # sample tiles 1-7 -> T1 s9-15
                tp(slot(T1, 8 + c), spackR[:, c - 1, :])
            for t in range(8):                           # rhs tiles 8-15 -> T1 s0-7
                tp(slot(T1, t), pk_h[1][:, t, :])
            # rhs drains ride two separate tile chains and overlap
            nc.vector.tensor_copy(rhs23[0:K, :], T1[0:K, 0:1024])
            nc.scalar.copy(s1T[0:K, 128:1024], T1[0:K, 1152:2048])

            # ---- main loop; chunk 0 split in halves so its first exp
            # starts as soon as rhs01 lands (the per-tile access chain
            # serializes mm/exp interleave safely in program order) ----
            dh = pp.tile([128, 2], f32)
            # chunk 0 halves on DIFFERENT PSUM buffers: half A in T0 cols
            # 0:1024, half B in T1 cols 1024:2048 (sample slots, long since
            # drained) — so expB's matmuls never sit behind expA's read in
            # the same tile chain.
            psA = psp.tile([128, M], f32, tag="mm")
            psB = psp.tile([128, M], f32, tag="mm")
            eo0 = xp.tile([128, M], f32, tag="eof", name="eof")
            mm(psA, 0, 0)
            mm(psA, 0, 1)
            nc.scalar.activation(eo0[:, 0:1024], psA[:, 0:1024], AF.Exp,
                                 bias=ebias[:], scale=1.0)
            nc.tensor.matmul(psB[:, 1024:1536], lhsT=s1T[0:K, 0:128],
                             rhs=rhs23[0:K, 0:512],
                             start=True, stop=True, skip_group_check=True)
            nc.tensor.matmul(psB[:, 1536:2048], lhsT=s1T[0:K, 0:128],
                             rhs=rhs23[0:K, 512:1024],
                             start=True, stop=True, skip_group_check=True)
            nc.scalar.activation(eo0[:, 1024:2048], psB[:, 1024:2048],
                                 AF.Exp, bias=ebias[:], scale=1.0)
            nc.vector.tensor_reduce(dh[:, 0:1], eo0[:, 0:1024],
                                    axis=AX.X, op=OP.add)
            nc.vector.tensor_reduce(dh[:, 1:2], eo0[:, 1024:2048],
                                    axis=AX.X, op=OP.add)
            nc.vector.tensor_reduce(dist_sb[:, 0:1], dh[:], axis=AX.X,
                                    op=OP.add)
            for c in range(1, CT):
                ps = psp.tile([128, M], f32, tag="mm")
                if c <= 4:
                    eo = xp.tile([128, M], f32, tag="eof", name="eof")
                else:
                    eo = xp.tile([128, M], bf16, tag="eo", name="eo")
                for j in range(4):
                    mm(ps, c, j)
                if c <= 4:
                    nc.scalar.activation(eo[:], ps[:], AF.Exp, bias=ebias[:],
                                         scale=1.0)
                    nc.vector.tensor_reduce(dist_sb[:, c:c + 1], eo[:],
                                            axis=AX.XY, op=OP.add)
                else:
                    nc.scalar.activation(eo[:], ps[:], AF.Exp, bias=ebias[:],
                                         scale=1.0, accum_out=dist_sb[:, c:c + 1])

            # output DMA split on SP: the bulk piece overlaps chunk 7's exp,
            # only the final element pays the post-transfer latency
            dist_ap = dist_d.ap().rearrange("(p c) -> p c", p=128)
            nc.sync.dma_start(dist_ap[:, 0:CT - 1], dist_sb[:, 0:CT - 1])
            nc.sync.dma_start(dist_ap[:, CT - 1:CT], dist_sb[:, CT - 1:CT])

    nc.compile()
    return nc


def _get_nc():
    if "nc" not in _CACHE:
        _CACHE["nc"] = _build_nc()
    return _CACHE["nc"]


def kernel(samples: np.ndarray, means: np.ndarray, stds: np.ndarray) -> np.ndarray:
    from concourse.bass_utils import run_bass_kernel_spmd

    samples = np.ascontiguousarray(samples, dtype=np.float32)
    means = np.ascontiguousarray(means, dtype=np.float32)
    stds = np.ascontiguousarray(stds, dtype=np.float32)

    nc = _get_nc()
    in_maps = [
        {"samples": samples[i * NSH:(i + 1) * NSH], "means": means, "stds": stds}
        for i in range(N_CORES)
    ]
    res = run_bass_kernel_spmd(nc, in_maps, list(range(N_CORES)))
    dist = np.concatenate([res.results[i]["dist"] for i in range(N_CORES)])
    return (-dist + dist.max() + dist.min()).astype(np.float32)
